# revision 69
# baseline (speedup 1.0000x reference)
"""Trainium2 Bass kernel for nn_AttnBlock (B=4, C=256, T=4096) on 8 NeuronCores.

Sharding: core = (batch b = core//2, query-half = core%2). Weights replicated.
Masked positions (~10%) are compacted away on the host: each core's column
list is [its own unmasked queries | pad | the other half's unmasked keys |
pad], so the kernel processes NQ query slots against NK key slots (both
mask-dependent, rounded up; the compiled program is cached per (NQ, NK)).
Attention is permutation-invariant over keys and masked-query outputs are
zero, so this is exact. Pad columns are zero with mask 0: k=q=v=0 there, and
the mask-weighted denominator excludes them.

fp8 fast path (~1.0e-2 max rel err vs the f32 reference, tolerance 2e-2):
attention matmuls are float8e4 DoubleRow (2 contraction tiles per
instruction, 0.5 PE cycles per moving row).

Key algebraic folds (all exact; biases/beta are zero, asserted):
  - gamma/beta fold into Wp; the LayerNorm mean-subtraction folds into
    centered projection weights Wc.  W' = W @ Wc for k/q/v.
  - Column scaling commutes through the 1x1 convs: the host pre-scales x
    columns by rstd * mask / 4, so scores = k^T q already carry the
    1/sqrt(C) = 1/16 softmax scale and exp needs only a constant bias
    (-SHIFT, fp8 range guard) which cancels between numerator/denominator.
  - The denominator is a mask-column DoubleRow ones-matmul on PE.
  - v drain multiplies by 4; q drain adds bq/4; out scaled by mask/denom.
  - v-bias and out-bias reduce to a host-side constant: (Wo @ bv + bo) * m.

Pipeline: PE p-state warmup during the DMA wait; pre-phase computes q-block0
+ k-blocks0,1 + all v chunks (psum borrowed from the score pool); then
NP-pair slots per query tile, paced by ACT's [128,2*W] exp; hpre/denominator
run two pairs behind; remaining k/q conv groups and the previous tile's
epilogue spread across slots on the single spare psum bank.
"""
import sys

if "/opt/trn_rl_repo" not in sys.path:
    sys.path.insert(0, "/opt/trn_rl_repo")

import numpy as np
import ml_dtypes

import concourse.tile as tile
from concourse import bacc, mybir
from concourse.bass_utils import run_bass_kernel_spmd
from concourse.alu_op_type import AluOpType as ALU

B, C, T = 4, 256, 4096
TH = T // 2
N_CORES = 8
EPS = 1e-5
SHIFT = 4.0          # global exp shift: e = exp(score - SHIFT) <= ~70 << 240

BF16 = mybir.dt.bfloat16
F32 = mybir.dt.float32
FP8 = mybir.dt.float8e4
NP_BF16 = ml_dtypes.bfloat16
NP_FP8 = ml_dtypes.float8_e4m3
AF = mybir.ActivationFunctionType
DR = mybir.MatmulPerfMode.DoubleRow


def _blocks(total, width):
    out = []
    off = 0
    while off < total:
        w = min(width, total - off)
        out.append((off, w))
        off += w
    return out


def build_kernel(NQ, NK):
    nc = bacc.Bacc("TRN2", target_bir_lowering=False, debug=False,
                   num_devices=N_CORES)
    NS = NK // 128
    d_x2 = nc.dram_tensor("x2", [128, 2, NK], BF16, kind="ExternalInput").ap()
    d_w = nc.dram_tensor("wcat", [128, 4, 2, 256], BF16,
                         kind="ExternalInput").ap()
    d_cols = nc.dram_tensor("cols", [128, 2 + NQ // 128], F32,
                            kind="ExternalInput").ap()
    d_m8 = nc.dram_tensor("m8d", [128, NS, 32], FP8,
                          kind="ExternalInput").ap()
    d_out = nc.dram_tensor("out", [NQ, C], BF16,
                           kind="ExternalOutput").ap()

    with tile.TileContext(nc) as tc:
        _body(tc, d_x2, d_w, d_cols, d_m8, d_out, NQ, NK)
    nc.compile()
    return nc


def _body(tc, d_x2, d_w, d_cols, d_m8, d_out, NQ, NK):
    nc = tc.nc
    from contextlib import ExitStack

    NS = NK // 128       # key chunks
    NP = NK // 256       # chunk pairs
    TW = [w for _, w in _blocks(NQ, 512)]     # query tile widths
    NTT = len(TW)
    toff = [o for o, _ in _blocks(NQ, 512)]

    with ExitStack() as ctx:
        consts = ctx.enter_context(tc.tile_pool(name="consts", bufs=1))
        big = ctx.enter_context(tc.tile_pool(name="big", bufs=1))

        x2 = consts.tile([128, 2, NK], BF16, tag="x2")
        x2_pieces = [(0, 512)] + _blocks(NK - 512, 544)
        x2_pieces = [(0, 512)] + [(o + 512, w) for o, w in x2_pieces[1:]]

        def load_x2(piece):
            o, w = x2_pieces[piece]
            pp = slice(o, o + w)
            nc.sync.dma_start(x2[:, :, pp], d_x2[:, :, pp])

        # The DMA fabric is one serial ~350B/ns resource: order transfers
        # strictly by first use; all triggers on the SP ring + SWDGE so the
        # ACT engine stays free for drains.
        wz = consts.tile([128, 512], BF16, tag="wz")
        nc.vector.memset(wz[:], 0.0)
        cols = consts.tile([128, 2 + NQ // 128], F32, tag="cols")
        nc.gpsimd.dma_start(cols[:], d_cols[:])
        wcat = consts.tile([128, 4, 2, 256], BF16, tag="wcat")
        nc.sync.dma_start(wcat[:, 0:2], d_w[:, 0:2])     # wk, wq first
        load_x2(0)
        load_x2(1)
        nc.gpsimd.dma_start(wcat[:, 2:4], d_w[:, 2:4])   # wv, wo (SWDGE)
        for piece in range(2, len(x2_pieces)):
            load_x2(piece)
        m8 = consts.tile([128, NS, 32], FP8, tag="m8")
        nc.gpsimd.dma_start(m8[:], d_m8[:])

        wk, wq, wv, wo = (wcat[:, i] for i in range(4))
        bq = cols[:, 0:2]
        mt = cols[:, 2:2 + NQ // 128]

        ones11 = consts.tile([1, 1], F32, tag="ones11")
        nc.vector.memset(ones11[:], 1.0)
        nshift = consts.tile([128, 1], F32, tag="nshift")
        nc.vector.memset(nshift[:], -SHIFT)
        ebase = consts.tile([128, 2, 512], F32, tag="ebase")
        nc.vector.memset(ebase[:], float(np.e))

        k_sb = big.tile([128, 2, NK], FP8, tag="k")
        q_sb = big.tile([128, 2, NQ], FP8, tag="q")
        vt_sb = big.tile([128, NS, 256], FP8, tag="vt")

        # PSUM: scp 2x2 banks, hpp 2, dnp 1, convp 1  (= 8)
        scp = ctx.enter_context(tc.tile_pool(name="scp", bufs=2,
                                             space="PSUM"))
        hpp = ctx.enter_context(tc.tile_pool(name="hpp", bufs=1,
                                             space="PSUM"))
        dnp = ctx.enter_context(tc.tile_pool(name="dnp", bufs=1,
                                             space="PSUM"))
        convp = ctx.enter_context(tc.tile_pool(name="convp", bufs=1,
                                               space="PSUM"))
        s1t = ctx.enter_context(tc.tile_pool(name="s1t", bufs=3))
        shp = ctx.enter_context(tc.tile_pool(name="shp", bufs=2))
        e_pool = ctx.enter_context(tc.tile_pool(name="e_pool", bufs=7))
        hso = ctx.enter_context(tc.tile_pool(name="hso", bufs=2))
        o_po = ctx.enter_context(tc.tile_pool(name="o_po", bufs=2))

        dn = dnp.tile([128, 512], F32, tag="dn")

        # warm the exp table while DMAs land
        dummy = s1t.tile([1, 1], F32, tag="dummy")
        nc.scalar.activation(dummy[:], ones11[:], AF.Exp, bias=0.0)

        # warm the PE p-state during the DMA wait
        wp = convp.tile([128, 512], F32, name="warm", tag="cv")
        for i in range(5):
            nc.tensor.matmul(wp[:], wz[:, 0:128], wz[:],
                             start=(i == 0), stop=(i == 4),
                             skip_group_check=True)

        # ---------- conv building blocks ----------
        rr = {"i": 0}

        def drain(out_ap, in_ap, kind="copy", arg=None, engines=(0, 1)):
            i = engines[rr["i"] % len(engines)]
            rr["i"] += 1
            if kind == "copy":
                if i == 0:
                    nc.vector.tensor_copy(out_ap, in_ap)
                else:
                    nc.scalar.copy(out_ap, in_ap)
            elif kind == "mul":
                if i == 0:
                    nc.vector.tensor_scalar_mul(out_ap, in_ap, arg)
                else:
                    nc.scalar.activation(out_ap, in_ap, AF.Copy, bias=0.0,
                                         scale=arg)
            elif kind == "bias":
                if i == 0:
                    nc.vector.tensor_scalar_add(out_ap, in_ap, arg)
                else:
                    nc.scalar.activation(out_ap, in_ap, AF.Identity, bias=arg)

        kq_blocks = {0: _blocks(NK, 512), 1: _blocks(NQ, 512)}

        def conv_k_hpp(j):
            # k block via the hpre banks (free until tile-0 hpre starts):
            # both cout halves at once, drains on DVE
            o, wd = kq_blocks[0][j]
            sl = slice(o, o + wd)
            pt = hpp.tile([128, 2, 512], F32, name="cvh", tag="hpre")
            for m in range(2):
                mm = slice(128 * m, 128 * (m + 1))
                nc.tensor.matmul(pt[:, m, 0:wd], wk[:, 0, mm], x2[:, 0, sl],
                                 start=True, stop=False,
                                 skip_group_check=True)
                nc.tensor.matmul(pt[:, m, 0:wd], wk[:, 1, mm], x2[:, 1, sl],
                                 start=False, stop=True,
                                 skip_group_check=True)
            nc.vector.tensor_copy(k_sb[:, 0, sl], pt[:, 0, 0:wd])
            nc.vector.tensor_copy(k_sb[:, 1, sl], pt[:, 1, 0:wd])

        def conv_kq(which, j, m, pre, engines=(0, 1)):
            """k (which=0) or q (which=1) column block j, cout half m."""
            w = wk if which == 0 else wq
            dst = k_sb if which == 0 else q_sb
            o, wd = kq_blocks[which][j]
            sl = slice(o, o + wd)
            mm = slice(128 * m, 128 * (m + 1))
            if pre:
                pt = scp.tile([128, 2, 512], F32, name="cvpre", tag="sc")
                p = pt[:, m % 2, 0:wd]
            else:
                pt = convp.tile([128, 512], F32, name="cv", tag="cv")
                p = pt[:, 0:wd]
            nc.tensor.matmul(p, w[:, 0, mm], x2[:, 0, sl],
                             start=True, stop=False, skip_group_check=True)
            nc.tensor.matmul(p, w[:, 1, mm], x2[:, 1, sl],
                             start=False, stop=True, skip_group_check=True)
            if which == 0:
                drain(dst[:, m, sl], p, "copy", engines=engines)
            else:
                drain(dst[:, m, sl], p, "bias", bq[:, m:m + 1],
                      engines=engines)

        def conv_v4(c0):
            """v chunks c0..c0+3 into one borrowed scp tile (pre-phase);
            drained with a single wide copy (same linear layout)."""
            pt = scp.tile([128, 2, 512], F32, name="v4", tag="sc")
            n = min(4, NS - c0)
            for i in range(n):
                c = c0 + i
                sl = slice(128 * c, 128 * (c + 1))
                p = pt[:, i // 2, 256 * (i % 2):256 * (i % 2) + 256]
                nc.tensor.matmul(p, x2[:, 0, sl], wv[:, 0],
                                 start=(i % 2 == 0), stop=False,
                                 skip_group_check=True)
                nc.tensor.matmul(p, x2[:, 1, sl], wv[:, 1],
                                 start=False, stop=(i % 2 == 1),
                                 skip_group_check=True)
            if n == 4:
                drain(vt_sb[:, c0:c0 + 4, :], pt[:], "mul", 4.0)
            else:
                for h in range((n + 1) // 2):
                    nn = min(2, n - 2 * h)
                    drain(vt_sb[:, c0 + 2 * h:c0 + 2 * h + nn, :],
                          pt[:, h, 0:256 * nn], "mul", 4.0)

        # ---------- attention building blocks ----------
        e_tiles = {}
        hpre_t = {}

        def scores_exp(jt, p, offload=False):
            W = TW[jt]
            h = W // 2
            sc = scp.tile([128, 2, 512], F32, tag="sc")
            for cpar in range(2):
                js = 2 * p + cpar
                lhs = k_sb[:, :, 128 * js:128 * js + 128]
                for th in range(2):
                    nc.tensor.matmul(
                        sc[:, cpar, h * th:h * th + h],
                        lhs,
                        q_sb[:, :, toff[jt] + h * th:toff[jt] + h * th + h],
                        start=True, stop=True, perf_mode=DR,
                        skip_group_check=True)
            e = e_pool.tile([128, 2, 512], FP8, tag="e")
            if offload:
                # exp on the otherwise-idle Pool engine: DVE moves the
                # psum scores to SBUF with the -SHIFT folded in, gpsimd
                # computes e^x as pow(e, x) (bit-exact vs ACT exp on HW)
                sh = shp.tile([128, 2, 512], F32, tag="sh")
                nc.vector.tensor_scalar_add(sh[:, :, 0:W], sc[:, :, 0:W],
                                            -SHIFT)
                nc.gpsimd.tensor_tensor(e[:, :, 0:W], ebase[:, :, 0:W],
                                        sh[:, :, 0:W], op=ALU.pow)
            else:
                nc.scalar.activation(e[:, :, 0:W], sc[:, :, 0:W], AF.Exp,
                                     bias=nshift[:, 0:1])
            e_tiles[(jt, p)] = e

        def hpre_dn(jt, p):
            W = TW[jt]
            h = W // 2
            e = e_tiles.pop((jt, p))
            hpre = hpre_t[jt]
            # start=True only on the first matmul touching each psum bank:
            # start marks the whole 2KB zero region pending, so the second
            # th-half's first write lands as a replace, then accumulates.
            for m in range(2):
                lhs = vt_sb[:, 2 * p:2 * p + 2, 128 * m:128 * m + 128]
                for th in range(2):
                    nc.tensor.matmul(
                        hpre[:, m, h * th:h * th + h],
                        lhs, e[:, :, h * th:h * th + h],
                        start=(p == 0 and th == 0), stop=(p == NP - 1),
                        perf_mode=DR, skip_group_check=True)
            for th in range(2):
                nc.tensor.matmul(
                    dn[0:32, h * th:h * th + h],
                    m8[:, 2 * p:2 * p + 2, :],
                    e[:, :, h * th:h * th + h],
                    start=(p == 0 and th == 0), stop=(p == NP - 1),
                    perf_mode=DR, skip_group_check=True)

        def hpre_drain(jt):
            W = TW[jt]
            hs = hso.tile([128, 2, 512], BF16, tag="hs")
            nc.vector.tensor_copy(hs[:, :, 0:W], hpre_t.pop(jt)[:, :, 0:W])
            return hs

        def dn_drow(jt, eng=0):
            W = TW[jt]
            drow = s1t.tile([1, 512], F32, tag="drow")
            if eng == 0:
                nc.vector.tensor_copy(drow[:, 0:W], dn[0:1, 0:W])
            else:
                nc.scalar.copy(drow[:, 0:W], dn[0:1, 0:W])
            return drow

        def dn_dcol(jt, drow):
            nc4 = TW[jt] // 128
            dcol = convp.tile([128, 4], F32, tag="cv")
            for c4 in range(nc4):
                nc.tensor.matmul(dcol[:, c4:c4 + 1],
                                 drow[0:1, 128 * c4:128 * (c4 + 1)],
                                 ones11[:], start=True, stop=True,
                                 skip_group_check=True)
            return dcol

        def fscale_of(jt, dcol):
            nc4 = TW[jt] // 128
            rinv = s1t.tile([128, 4], F32, tag="rinv")
            nc.vector.reciprocal(rinv[:, 0:nc4], dcol[:, 0:nc4])
            fs = s1t.tile([128, 4], F32, tag="fs")
            nc.vector.tensor_mul(fs[:, 0:nc4], rinv[:, 0:nc4],
                                 mt[:, toff[jt] // 128:
                                    toff[jt] // 128 + nc4])
            return fs

        def epi_mm(jt, c4, hs, o_sb, fs, last=False):
            cs = slice(128 * c4, 128 * (c4 + 1))
            if last:
                ott = scp.tile([128, 2, 512], F32, name="otl", tag="sc")
                ot = ott[:, 0, 0:256]
            else:
                ot = convp.tile([128, 256], F32, name="cv", tag="cv")
            nc.tensor.matmul(ot, hs[:, 0, cs], wo[:, 0],
                             start=True, stop=False, skip_group_check=True)
            nc.tensor.matmul(ot, hs[:, 1, cs], wo[:, 1],
                             start=False, stop=True, skip_group_check=True)
            drain(o_sb[:, c4], ot, "mul", fs[:, c4:c4 + 1],
                  engines=(0,) if not last else (0, 1))

        def out_dma(jt, o_sb):
            nc4 = TW[jt] // 128
            dview = d_out[toff[jt]:toff[jt] + TW[jt], :] \
                .rearrange("(c p) o -> p c o", p=128)
            nc.sync.dma_start(dview, o_sb[:, 0:nc4])

        # ---------------- pre-phase ----------------
        conv_kq(1, 0, 0, True)   # q block 0 (tile 0)
        conv_kq(1, 0, 1, True)
        conv_kq(0, 0, 0, True)   # k blocks 0,1 (pairs 0..3)
        conv_kq(0, 0, 1, True)
        conv_v4(0)
        conv_kq(0, 1, 0, True)
        conv_kq(0, 1, 1, True)
        for c0 in range(4, NS, 4):
            conv_v4(c0)

        # conv/epilogue work spread into pair slots
        kq_sched = {}
        for j in range(2, len(kq_blocks[0])):
            for m in range(2):
                kq_sched.setdefault((0, 2 * (j - 2) + m), []).append((0, j, m))
        qslots = [(0, 12), (0, 13), (1, 0), (1, 1), (2, 0), (2, 1),
                  (1, 5), (1, 6)]
        qi = 0
        for j in range(1, len(kq_blocks[1])):
            for m in range(2):
                kq_sched.setdefault(qslots[qi], []).append((1, j, m))
                qi += 1

        # ---------------- attention ----------------
        st = {}

        def tile_epilogue_step(jt, p, prev):
            if p == 3:
                st["dcol"] = dn_dcol(prev, st.pop("drow"))
            elif p == 6:
                st["fs"] = fscale_of(prev, st.pop("dcol"))
                st["o_sb"] = o_po.tile([128, 4, 256], BF16, name="o_sb",
                                       tag="o_sb")
            elif p in (7, 9, 11, 13):
                c4 = (p - 7) // 2
                if c4 < TW[prev] // 128:
                    epi_mm(prev, c4, st["hs"], st["o_sb"], st["fs"])
            elif p == 14:
                out_dma(prev, st.pop("o_sb"))
                st.pop("hs")
                st.pop("fs")

        for jt in range(NTT):
            for p in range(NP):
                scores_exp(jt, p)
                if jt == 0:
                    if p == 2:
                        hpre_t[jt] = hpp.tile([128, 2, 512], F32,
                                              name="hpre", tag="hpre")
                    if p >= 2:
                        hpre_dn(jt, p - 2)
                else:
                    if p == 0:
                        hpre_dn(jt - 1, NP - 2)
                    elif p == 1:
                        hpre_dn(jt - 1, NP - 1)
                    elif p == 2:
                        st["hs"] = hpre_drain(jt - 1)
                        st["drow"] = dn_drow(jt - 1)
                    elif p == 3:
                        hpre_t[jt] = hpp.tile([128, 2, 512], F32,
                                              name="hpre", tag="hpre")
                        hpre_dn(jt, 0)
                        hpre_dn(jt, 1)
                    else:
                        hpre_dn(jt, p - 2)
                for item in kq_sched.get((jt, p), ()):
                    which, j, m = item
                    conv_kq(which, j, m, False, engines=(0,))
                if jt > 0:
                    tile_epilogue_step(jt, p, jt - 1)

        # ---------------- tail: last tile ----------------
        jt = NTT - 1
        W = TW[jt]
        nc4 = W // 128
        hpre_dn(jt, NP - 2)
        hpre_dn(jt, NP - 1)
        hs = hso.tile([128, 2, 512], BF16, tag="hs")
        hp = hpre_t.pop(jt)
        drow = dn_drow(jt, eng=1)                    # ACT is idle here
        nc.vector.tensor_copy(hs[:, 0, 0:W], hp[:, 0, 0:W])
        nc.scalar.copy(hs[:, 1, 0:W], hp[:, 1, 0:W])
        dcol = dn_dcol(jt, drow)
        fs = fscale_of(jt, dcol)
        o_sb = o_po.tile([128, 4, 256], BF16, tag="o_sb")
        dv = d_out[toff[jt]:toff[jt] + W, :].rearrange("(c p) o -> p c o",
                                                       p=128)
        for c4 in range(nc4):
            epi_mm(jt, c4, hs, o_sb, fs, last=(c4 < nc4 - 1))
            if c4 > 0:
                ring = nc.sync if c4 % 2 else nc.scalar
                ring.dma_start(dv[:, c4 - 1:c4], o_sb[:, c4 - 1:c4])
        nc.scalar.dma_start(dv[:, nc4 - 1:nc4], o_sb[:, nc4 - 1:nc4])


_NC_CACHE = {}


def _get_nc(NQ, NK):
    key = (NQ, NK)
    if key not in _NC_CACHE:
        _NC_CACHE[key] = build_kernel(NQ, NK)
    return _NC_CACHE[key]


def _chunk_pf(a, last, dt=NP_BF16):
    """[256, last] -> [128, 2, last] partition-first."""
    return np.ascontiguousarray(
        a.astype(dt).reshape(2, 128, last).transpose(1, 0, 2))


def _prep_shared(gamma, beta, Wp, bp, Wq, bq, Wk, bk, Wv, bv, Wo, bo):
    # bk and the post-Wp constant (Wp@beta + bp) cannot fold through the
    # prescale trick; both are zero for this problem's setup_inputs.
    assert not np.any(bk), "nonzero bk not supported by this kernel"
    assert not np.any(bp + Wp @ beta), \
        "nonzero bp/beta not supported by this kernel"
    Wp_g = (Wp * gamma[None, :]).astype(np.float32)
    ws = Wp_g.sum(axis=1)
    Wc = Wp_g - ws[:, None] / C                        # centered W~^T [o, c]
    wcat = np.stack([_chunk_pf((Wk @ Wc).T, 256),
                     _chunk_pf((Wq @ Wc).T, 256),
                     _chunk_pf((Wv @ Wc).T, 256),
                     _chunk_pf(Wo.T, 256)], axis=1)    # [128, 4, 2, 256]
    shared = {
        "wcat": np.ascontiguousarray(wcat),
        "bq_col": np.ascontiguousarray(
            (bq.astype(np.float32) / 4.0).reshape(2, 128).T),
    }
    const_vec = Wo @ bv + bo                           # host-side bias
    return shared, const_vec


def _roundup(x, k):
    return -(-x // k) * k


def kernel(x, x_mask, gamma, beta, Wp, bp, Wq, bq, Wk, bk, Wv, bv, Wo, bo):
    x = np.asarray(x, np.float32)
    m = np.asarray(x_mask, np.float32)
    args = [np.asarray(a, np.float32) for a in
            (gamma, beta, Wp, bp, Wq, bq, Wk, bk, Wv, bv, Wo, bo)]
    shared, const_vec = _prep_shared(*args)

    # LayerNorm stats on the host (O(C*T) fp32), folded into the x columns
    var = x.var(axis=1)
    rstd_b = 1.0 / np.sqrt(var + EPS)                  # [B, T]
    colscale = rstd_b * m[:, 0, :] * 0.25              # [B, T]
    mb = m[:, 0, :] > 0.5

    # compaction: per-core column list = own unmasked queries | pad |
    # other half's unmasked keys | pad
    own_l, oth_l = [], []
    for core in range(N_CORES):
        b, half = divmod(core, 2)
        own_l.append(np.nonzero(mb[b, half * TH:(half + 1) * TH])[0]
                     + half * TH)
        oth_l.append(np.nonzero(mb[b, (1 - half) * TH:(2 - half) * TH])[0]
                     + (1 - half) * TH)
    NQ = _roundup(max(len(o) for o in own_l), 128)
    NK = _roundup(NQ + max(len(o) for o in oth_l), 256)
    # the slot schedule assumes >=3 query tiles and >=15 chunk pairs;
    # pad up for unusually heavy masks (extra columns are zero/masked)
    NQ = max(NQ, 1536)
    NK = max(NK, NQ + 256, 15 * 256)

    in_maps = []
    for core in range(N_CORES):
        b = core // 2
        own, oth = own_l[core], oth_l[core]
        xs = x[b] * colscale[b][None, :]               # [C, T] scaled
        xr = np.zeros((C, NK), np.float32)
        xr[:, :len(own)] = xs[:, own]
        xr[:, NQ:NQ + len(oth)] = xs[:, oth]
        mk = np.zeros(NK, np.float32)                  # real-column mask
        mk[:len(own)] = 1.0
        mk[NQ:NQ + len(oth)] = 1.0
        cols = np.empty((128, 2 + NQ // 128), np.float32)
        cols[:, 0:2] = shared["bq_col"]
        cols[:, 2:] = mk[:NQ].reshape(NQ // 128, 128).T
        m8d = np.broadcast_to(
            mk.astype(NP_FP8).reshape(NK // 128, 128).T[:, :, None],
            (128, NK // 128, 32))
        in_maps.append({
            "wcat": shared["wcat"],
            "x2": _chunk_pf(xr, NK),
            "cols": np.ascontiguousarray(cols),
            "m8d": np.ascontiguousarray(m8d),
        })

    nc = _get_nc(NQ, NK)
    res = run_bass_kernel_spmd(nc, in_maps, list(range(N_CORES)))

    out = np.zeros((B, C, T), np.float32)
    for core in range(N_CORES):
        b = core // 2
        own = own_l[core]
        out[b][:, own] = res.results[core]["out"][:len(own)]\
            .astype(np.float32).T
    out += (x + const_vec[None, :, None]) * m
    return out
